# revision 24
# baseline (speedup 1.0000x reference)
"""Trainium2 kernel for nn_PennyLaneQuantumClassifier.

Math: the quantum circuit is linear in the state vector, and the state is
amplitude-encoded from only N_INPUTS=10 real amplitudes.  Hence the PauliZ
expectation collapses to a quadratic form

    z0 = xs^T A xs / (xs^T xs),       xs = tanh(x * scale)

with A a 10x10 real symmetric matrix depending only on theta.  Using the
eigendecomposition A = V diag(lam) V^T (V orthogonal):

    g  = V^T xs
    t_j = sum((lam*w_j + b_j) * g^2)   (j = 0, 1)
    s   = sum(g^2)                      (= |xs|^2, V orthogonal)
    out_j = t_j / s

The device kernel (raw bacc, manual semaphores) streams x in a
feature-on-partition packed layout (8 row-chunks of 10 features stacked on
80 partitions, scale prepended as column 0 of the x DMA).  Per column tile:
ACT tanh -> PE matvec (block-diag V, f32r) -> ACT square -> PE reduction
matmul (s dups at rows 0-15, t0/t1 at rows 64-79) -> ACT copy of the t
rows to SBUF -> one fused custom-DVE divide (seed + 1 Newton pass + the
multiply, in0 = s psum rows at base 0, in1 = copied t rows) -> one
output DMA per tile (component-major; host interleaves the two output
columns during the gather).  Pure data-parallel across 8 NeuronCores.
"""

import numpy as np

N_QUBITS = 10
N_LAYERS = 4
N_INPUTS = 10
DIM = 2**N_QUBITS

BATCH = 32768
NCORES = 8
ROWS = BATCH // NCORES          # 4096 rows per core
C = 8                           # row-chunks stacked on partitions
NCOL = ROWS // C                # 512 columns (rows per chunk)
P = C * N_INPUTS                # 80 partitions used
NCONST = 1 + P + 96             # scale | bdv | red columns

T = 2                           # column tiles per core
END_WAIT = False                 # explicit wait for output DMA completion

_PROG_CACHE: dict = {}


def _divide_op():
    """out = in1 * approx-recip(in0): BITWISE_NOT seed + one Newton pass +
    the multiply fused into a single 6-stage custom-DVE op (~1.7e-3 max rel
    err).  Registered into the dve_ops tables on first use."""
    import re
    import concourse.dve_ops as dv
    from concourse.dve_spec import AluOp, Bin, C0, C1, Spec, Src0, Src1
    from concourse.dve_table_gen import dve_ver_for

    if "op" in _DIV_CACHE:
        return _DIV_CACHE["op"]
    not_x = Bin(AluOp.BITWISE_NOT, Src0, Src0)
    y0 = not_x * C0
    y1 = y0 * (C1 - Src0 * y0)

    def ref(in0, in1, c0, c1, c2):
        nx = (~in0.view(np.int32)).view(np.float32)
        r0 = nx * c0
        r1 = r0 * (c1 - in0 * r0)
        return r1 * in1

    spec = Spec(body=y1 * Src1, reference=ref)
    op = dv.DveOp(
        "DIVIDE_APPROX_ANT", spec, subdim=False,
        uops_sha={"v3": "e11870b101db7dce"},
    )
    if op.name not in dv._SUB_OPCODE_FOR_NAME:
        dv.OPS.append(op)
        dv._SUB_OPCODE_FOR_NAME[op.name] = (
            max(dv._SUB_OPCODE_FOR_NAME.values()) + 1
        )
        dv.CUSTOM_DVE_SPECS[op.name] = spec
    ver = dve_ver_for("TRN2")
    try:
        op.compile(ver)
    except ValueError as e:
        m = re.search(r"([0-9a-f]{16})", str(e))
        if not m:
            raise
        op.uops_sha[ver] = m.group(1)
        op.compile(ver)
    _DIV_CACHE["op"] = op
    return op


_DIV_CACHE: dict = {}


def _compute_A(theta: np.ndarray) -> np.ndarray:
    """Collapse the circuit: A[i,j] s.t. z0 = e^T A e for the embedded state."""
    th = theta.astype(np.float64).reshape(N_LAYERS, N_QUBITS, 3)
    a, b, c = th[..., 0], th[..., 1], th[..., 2]
    cb, sb = np.cos(b / 2), np.sin(b / 2)
    e = lambda t: np.exp(1j * t)
    u00 = e(-(a + c) / 2) * cb
    u01 = -1j * e((a - c) / 2) * sb
    u10 = -1j * e(-(a - c) / 2) * sb
    u11 = e((a + c) / 2) * cb
    U = np.stack([np.stack([u00, u01], -1), np.stack([u10, u11], -1)], -2)

    M = np.zeros((DIM, N_INPUTS), np.complex128)
    for i in range(N_INPUTS):
        M[i, i] = 1.0
    for l in range(N_LAYERS):
        for q in range(N_QUBITS):
            p = M.reshape(2**q, 2, -1, N_INPUTS)
            M = np.einsum("ab,qbri->qari", U[l, q], p).reshape(DIM, N_INPUTS)
        for q in range(N_QUBITS - 1):
            p = M.reshape(2**q, 2, 2, -1, N_INPUTS).copy()
            p[:, 1] = p[:, 1, ::-1]
            M = p.reshape(DIM, N_INPUTS)
    signs = np.concatenate([np.ones(DIM // 2), -np.ones(DIM // 2)])
    return np.real(M.conj().T @ (signs[:, None] * M))


def _act_reciprocal(nc, mybir, out, in_):
    """ACT Reciprocal without the bass accuracy guard (validated on HW)."""
    eng = nc.scalar
    return eng.add_instruction(
        mybir.InstActivation(
            name=nc.get_next_instruction_name(),
            func=mybir.ActivationFunctionType.Reciprocal,
            ins=[
                eng.lower_ap(in_),
                mybir.ImmediateValue(dtype=mybir.dt.float32, value=0.0),
                mybir.ImmediateValue(dtype=mybir.dt.float32, value=1.0),
                mybir.ImmediateValue(dtype=mybir.dt.float32, value=0.0),
            ],
            outs=[eng.lower_ap(out)],
        )
    )


def _build_program():
    import concourse.bacc as bacc
    import concourse.mybir as mybir
    from contextlib import ExitStack

    f32 = mybir.dt.float32
    f32r = mybir.dt.float32r
    WS = [256, 256]
    OFF = [0, 256]
    Tanh = mybir.ActivationFunctionType.Tanh
    Square = mybir.ActivationFunctionType.Square

    nc = bacc.Bacc(trn_type="TRN2", target_bir_lowering=False, debug=False)
    x_d = nc.dram_tensor("xp", [P, NCOL + 1], f32, kind="ExternalInput").ap()
    vr_d = nc.dram_tensor("vr", [P, P + P], f32r, kind="ExternalInput").ap()
    op_d = nc.dram_tensor("outp", [2 * C, NCOL], f32, kind="ExternalOutput").ap()

    xt = nc.alloc_sbuf_tensor("xt_raw", [P, NCOL + 1], f32).ap()
    vr_t = nc.alloc_sbuf_tensor("vr_raw", [P, P + P], f32r).ap()
    sc_ap = xt[:, 0:1]
    v_ap = vr_t[:, 0:P]
    r_ap = vr_t[:, P : P + P]
    xs = [nc.alloc_sbuf_tensor(f"xs{t}", [P, WS[t]], f32r).ap() for t in range(T)]
    h = [nc.alloc_sbuf_tensor(f"h{t}", [P, WS[t]], f32r).ap() for t in range(T)]
    ss = [nc.alloc_sbuf_tensor(f"ss{t}", [2 * C, WS[t]], f32).ap() for t in range(T)]
    rs = [nc.alloc_sbuf_tensor(f"rs{t}", [2 * C, WS[t]], f32).ap() for t in range(T)]
    o = [nc.alloc_sbuf_tensor(f"o{t}", [2 * C, WS[t]], f32).ap() for t in range(T)]

    in_x = nc.alloc_semaphore("in_x")
    in_sc = nc.alloc_semaphore("in_sc")
    in_vr = nc.alloc_semaphore("in_vr")
    out_sem = nc.alloc_semaphore("out_dma")
    act_sem = nc.alloc_semaphore("act")
    pe_sem = nc.alloc_semaphore("pe")
    dve_sem = nc.alloc_semaphore("dve")

    with ExitStack() as ctx:
        g = [
            ctx.enter_context(nc.psum_tensor(f"g{t}", [P, WS[t]], f32)).ap()
            for t in range(T)
        ]
        qs = [
            ctx.enter_context(nc.psum_tensor(f"qs{t}", [P, WS[t]], f32)).ap()
            for t in range(T)
        ]

        # SP: x half-tile DMA triggers (parallel HW queues), then gated
        # output DMAs (compact per-component halves; host interleaves)
        nc.sync.dma_start(
            xt[:, 0 : WS[0] + 1], x_d[:, 0 : WS[0] + 1]
        ).then_inc(in_x, 16)
        nc.sync.dma_start(
            xt[:, WS[0] + 1 : NCOL + 1], x_d[:, WS[0] + 1 : NCOL + 1]
        ).then_inc(in_sc, 16)
        for t in range(T):
            nc.sync.dma_start(
                op_d[:, OFF[t] : OFF[t] + WS[t]], o[t]
            )._wait_ge(dve_sem, t + 1).then_inc(out_sem, 16)
        if END_WAIT:
            nc.sync.wait_ge(out_sem, 32)

        # ACT: scale + weights DMAs on the second HWDGE engine, table
        # warm-up, tanh, square, s-copy.  act_sem counts from memzero.
        nc.scalar.dma_start(vr_t, vr_d).then_inc(in_vr, 16)
        nc.scalar.activation(
            xs[0], xt[:, 1 : WS[0] + 1], Tanh, scale=sc_ap
        )._wait_ge(in_x, 16).then_inc(act_sem, 1)  # act 1
        nc.scalar.activation(
            xs[1], xt[:, WS[0] + 1 : NCOL + 1], Tanh, scale=sc_ap
        )._wait_ge(in_sc, 16).then_inc(act_sem, 1)  # act 2
        for t in range(T):
            nc.scalar.activation(h[t], g[t], Square)._wait_ge(
                pe_sem, t + 1
            ).then_inc(act_sem, 1)  # act 3, 4
        # the copy now moves the t rows (psum partitions 64-79, proven
        # shifted-copy shape) while DVE reads the s rows at base 0
        for t in range(T):
            nc.scalar.copy(ss[t], qs[t][64 : 64 + 2 * C, :])._wait_ge(
                pe_sem, 3 + t
            ).then_inc(act_sem, 1)  # act 5, 6

        # PE: two matvecs, two reductions
        nc.tensor.wait_ge(in_vr, 16)
        for t in range(T):
            nc.tensor.matmul(
                g[t], v_ap, xs[t], start=True, stop=True
            )._wait_ge(act_sem, t + 1).then_inc(pe_sem, 1)  # pe 1, 2
        for t in range(T):
            nc.tensor.matmul(
                qs[t], r_ap, h[t], start=True, stop=True
            )._wait_ge(act_sem, 3 + t).then_inc(pe_sem, 1)  # pe 3, 4

        # DVE: one fused divide per tile — in0 = s rows straight off psum
        # (partitions 0-15, base 0), in1 = the ACT-copied t rows (whose
        # act gate also implies the reduction matmul is done)
        div = _divide_op()
        for t in range(T):
            nc.vector._custom_dve(
                div, out=o[t], in0=qs[t][0 : 2 * C, :], in1=ss[t],
                s0=-0.23549792, s1=2.0017324, imm2=0.0,
            )._wait_ge(act_sem, 5 + t).then_inc(dve_sem, 1)  # dve 1, 2

        nc.compile()
    return nc


def _get_program():
    if "nc" not in _PROG_CACHE:
        _PROG_CACHE["nc"] = _build_program()
    return _PROG_CACHE["nc"]


def _host_constants(scale, theta, out_w, out_b):
    A = _compute_A(np.asarray(theta))
    lam, V = np.linalg.eigh(A)
    w = np.asarray(out_w, np.float64)[:, 0]
    b = np.asarray(out_b, np.float64)

    scale_p = np.tile(np.asarray(scale, np.float64), C)[:, None]
    vr = np.zeros((P, P + P), np.float64)
    vr[:, 0:P] = np.kron(np.eye(C), V)
    for c in range(C):
        rows = slice(c * N_INPUTS, (c + 1) * N_INPUTS)
        vr[rows, P + c] = 1.0
        vr[rows, P + C + c] = 1.0
        vr[rows, P + 64 + c] = lam * w[0] + b[0]
        vr[rows, P + 64 + C + c] = lam * w[1] + b[1]
    return (np.ascontiguousarray(scale_p.astype(np.float32)),
            np.ascontiguousarray(vr.astype(np.float32)))


def kernel(x, scale, theta, out_w, out_b, _trace=False):
    from concourse.bass_utils import run_bass_kernel_spmd

    x = np.ascontiguousarray(np.asarray(x, np.float32))
    scale_p, vr = _host_constants(scale, theta, out_w, out_b)

    in_maps = []
    for k in range(NCORES):
        xc = x[k * ROWS : (k + 1) * ROWS]
        xp = xc.reshape(C, NCOL, N_INPUTS).transpose(0, 2, 1).reshape(P, NCOL)
        xp = np.ascontiguousarray(np.concatenate([scale_p, xp], axis=1))
        in_maps.append({"xp": xp, "vr": vr})

    nc = _get_program()
    res = run_bass_kernel_spmd(
        nc, in_maps, core_ids=list(range(NCORES)), trace=_trace
    )
    parts = []
    for k in range(NCORES):
        op = res.results[k]["outp"]
        parts.append(np.stack([op[0:C].reshape(ROWS), op[C:].reshape(ROWS)], -1))
    out = np.concatenate(parts, axis=0)
    if _trace:
        return out, res
    return out



# revision 26
# speedup vs baseline: 1.0742x; 1.0742x over previous
"""Trainium2 kernel for nn_PennyLaneQuantumClassifier.

Math: the quantum circuit is linear in the state vector, and the state is
amplitude-encoded from only N_INPUTS=10 real amplitudes.  Hence the PauliZ
expectation collapses to a quadratic form

    z0 = xs^T A xs / (xs^T xs),       xs = tanh(x * scale)

with A a 10x10 real symmetric matrix depending only on theta.  Using the
eigendecomposition A = V diag(lam) V^T (V orthogonal):

    g  = V^T xs
    t_j = sum((lam*w_j + b_j) * g^2)   (j = 0, 1)
    s   = sum(g^2)                      (= |xs|^2, V orthogonal)
    out_j = t_j / s

The device kernel (raw bacc, manual semaphores) streams x in a
feature-on-partition packed layout (8 row-chunks of 10 features stacked on
80 partitions, scale prepended as column 0 of the x DMA).  Per column tile:
ACT tanh -> PE matvec (block-diag V, f32r) -> ACT square -> PE reduction
matmul (s dups at rows 0-15, t0/t1 at rows 64-79) -> DVE 1-pass reciprocal
straight off the s psum rows concurrent with the ACT copy of the t rows to
SBUF -> one paired DVE multiply -> one
output DMA per tile (component-major; host interleaves the two output
columns during the gather).  Pure data-parallel across 8 NeuronCores.
"""

import numpy as np

N_QUBITS = 10
N_LAYERS = 4
N_INPUTS = 10
DIM = 2**N_QUBITS

BATCH = 32768
NCORES = 8
ROWS = BATCH // NCORES          # 4096 rows per core
C = 8                           # row-chunks stacked on partitions
NCOL = ROWS // C                # 512 columns (rows per chunk)
P = C * N_INPUTS                # 80 partitions used
NCONST = 1 + P + 96             # scale | bdv | red columns

T = 2                           # column tiles per core
END_WAIT = False                 # explicit wait for output DMA completion

_PROG_CACHE: dict = {}


def _compute_A(theta: np.ndarray) -> np.ndarray:
    """Collapse the circuit: A[i,j] s.t. z0 = e^T A e for the embedded state."""
    th = theta.astype(np.float64).reshape(N_LAYERS, N_QUBITS, 3)
    a, b, c = th[..., 0], th[..., 1], th[..., 2]
    cb, sb = np.cos(b / 2), np.sin(b / 2)
    e = lambda t: np.exp(1j * t)
    u00 = e(-(a + c) / 2) * cb
    u01 = -1j * e((a - c) / 2) * sb
    u10 = -1j * e(-(a - c) / 2) * sb
    u11 = e((a + c) / 2) * cb
    U = np.stack([np.stack([u00, u01], -1), np.stack([u10, u11], -1)], -2)

    M = np.zeros((DIM, N_INPUTS), np.complex128)
    for i in range(N_INPUTS):
        M[i, i] = 1.0
    for l in range(N_LAYERS):
        for q in range(N_QUBITS):
            p = M.reshape(2**q, 2, -1, N_INPUTS)
            M = np.einsum("ab,qbri->qari", U[l, q], p).reshape(DIM, N_INPUTS)
        for q in range(N_QUBITS - 1):
            p = M.reshape(2**q, 2, 2, -1, N_INPUTS).copy()
            p[:, 1] = p[:, 1, ::-1]
            M = p.reshape(DIM, N_INPUTS)
    signs = np.concatenate([np.ones(DIM // 2), -np.ones(DIM // 2)])
    return np.real(M.conj().T @ (signs[:, None] * M))


def _act_reciprocal(nc, mybir, out, in_):
    """ACT Reciprocal without the bass accuracy guard (validated on HW)."""
    eng = nc.scalar
    return eng.add_instruction(
        mybir.InstActivation(
            name=nc.get_next_instruction_name(),
            func=mybir.ActivationFunctionType.Reciprocal,
            ins=[
                eng.lower_ap(in_),
                mybir.ImmediateValue(dtype=mybir.dt.float32, value=0.0),
                mybir.ImmediateValue(dtype=mybir.dt.float32, value=1.0),
                mybir.ImmediateValue(dtype=mybir.dt.float32, value=0.0),
            ],
            outs=[eng.lower_ap(out)],
        )
    )


def _build_program():
    import concourse.bacc as bacc
    import concourse.mybir as mybir
    from contextlib import ExitStack

    f32 = mybir.dt.float32
    f32r = mybir.dt.float32r
    WS = [256, 256]
    OFF = [0, 256]
    Tanh = mybir.ActivationFunctionType.Tanh
    Square = mybir.ActivationFunctionType.Square

    nc = bacc.Bacc(trn_type="TRN2", target_bir_lowering=False, debug=False)
    x_d = nc.dram_tensor("xp", [P, NCOL + 1], f32, kind="ExternalInput").ap()
    vr_d = nc.dram_tensor("vr", [P, P + P], f32r, kind="ExternalInput").ap()
    op_d = nc.dram_tensor("outp", [2 * C, NCOL], f32, kind="ExternalOutput").ap()

    xt = nc.alloc_sbuf_tensor("xt_raw", [P, NCOL + 1], f32).ap()
    vr_t = nc.alloc_sbuf_tensor("vr_raw", [P, P + P], f32r).ap()
    sc_ap = xt[:, 0:1]
    v_ap = vr_t[:, 0:P]
    r_ap = vr_t[:, P : P + P]
    xs = [nc.alloc_sbuf_tensor(f"xs{t}", [P, WS[t]], f32r).ap() for t in range(T)]
    h = [nc.alloc_sbuf_tensor(f"h{t}", [P, WS[t]], f32r).ap() for t in range(T)]
    ss = [nc.alloc_sbuf_tensor(f"ss{t}", [2 * C, WS[t]], f32).ap() for t in range(T)]
    rs = [nc.alloc_sbuf_tensor(f"rs{t}", [2 * C, WS[t]], f32).ap() for t in range(T)]
    o = [nc.alloc_sbuf_tensor(f"o{t}", [2 * C, WS[t]], f32).ap() for t in range(T)]

    in_x = nc.alloc_semaphore("in_x")
    in_sc = nc.alloc_semaphore("in_sc")
    in_vr = nc.alloc_semaphore("in_vr")
    out_sem = nc.alloc_semaphore("out_dma")
    act_sem = nc.alloc_semaphore("act")
    pe_sem = nc.alloc_semaphore("pe")
    dve_sem = nc.alloc_semaphore("dve")

    with ExitStack() as ctx:
        g = [
            ctx.enter_context(nc.psum_tensor(f"g{t}", [P, WS[t]], f32)).ap()
            for t in range(T)
        ]
        qs = [
            ctx.enter_context(nc.psum_tensor(f"qs{t}", [P, WS[t]], f32)).ap()
            for t in range(T)
        ]

        # SP: x half-tile DMA triggers (parallel HW queues), then gated
        # output DMAs (compact per-component halves; host interleaves)
        nc.sync.dma_start(
            xt[:, 0 : WS[0] + 1], x_d[:, 0 : WS[0] + 1]
        ).then_inc(in_x, 16)
        nc.sync.dma_start(vr_t, vr_d).then_inc(in_vr, 16)
        for t in range(T):
            nc.sync.dma_start(
                op_d[:, OFF[t] : OFF[t] + WS[t]], o[t]
            )._wait_ge(dve_sem, 2 * (t + 1)).then_inc(out_sem, 16)
        if END_WAIT:
            nc.sync.wait_ge(out_sem, 32)

        # ACT: x half-2 rides this ring so both x halves land in
        # parallel; the weights moved to the SP ring as its second
        # transfer (still ahead of the first matvec's need).
        nc.scalar.dma_start(
            xt[:, WS[0] + 1 : NCOL + 1], x_d[:, WS[0] + 1 : NCOL + 1]
        ).then_inc(in_sc, 16)
        nc.scalar.activation(
            xs[0], xt[:, 1 : WS[0] + 1], Tanh, scale=sc_ap
        )._wait_ge(in_x, 16).then_inc(act_sem, 1)  # act 1
        nc.scalar.activation(
            xs[1], xt[:, WS[0] + 1 : NCOL + 1], Tanh, scale=sc_ap
        )._wait_ge(in_sc, 16).then_inc(act_sem, 1)  # act 2
        for t in range(T):
            nc.scalar.activation(h[t], g[t], Square)._wait_ge(
                pe_sem, t + 1
            ).then_inc(act_sem, 1)  # act 3, 4
        # the copy now moves the t rows (psum partitions 64-79, proven
        # shifted-copy shape) while DVE reads the s rows at base 0
        for t in range(T):
            nc.scalar.copy(ss[t], qs[t][64 : 64 + 2 * C, :])._wait_ge(
                pe_sem, 3 + t
            ).then_inc(act_sem, 1)  # act 5, 6

        # PE: two matvecs, two reductions
        nc.tensor.wait_ge(in_vr, 16)
        for t in range(T):
            nc.tensor.matmul(
                g[t], v_ap, xs[t], start=True, stop=True
            )._wait_ge(act_sem, t + 1).then_inc(pe_sem, 1)  # pe 1, 2
        for t in range(T):
            nc.tensor.matmul(
                qs[t], r_ap, h[t], start=True, stop=True
            )._wait_ge(act_sem, 3 + t).then_inc(pe_sem, 1)  # pe 3, 4

        # DVE: reciprocal straight off the s rows (psum partitions 0-15)
        # concurrent with the ACT t-copy, then SBUF-x-SBUF multiply
        for t in range(T):
            nc.vector.reciprocal_approx_fast(
                out=rs[t], in_=qs[t][0 : 2 * C, :]
            )._wait_ge(pe_sem, 3 + t).then_inc(dve_sem, 1)  # dve 1, 3
            nc.vector.wait_ge(act_sem, 5 + t)
            nc.vector.tensor_mul(
                o[t], ss[t], rs[t]
            )._wait_ge(dve_sem, 2 * t + 1).then_inc(dve_sem, 1)  # dve 2, 4

        nc.compile()
    return nc


def _get_program():
    if "nc" not in _PROG_CACHE:
        _PROG_CACHE["nc"] = _build_program()
    return _PROG_CACHE["nc"]


def _host_constants(scale, theta, out_w, out_b):
    A = _compute_A(np.asarray(theta))
    lam, V = np.linalg.eigh(A)
    w = np.asarray(out_w, np.float64)[:, 0]
    b = np.asarray(out_b, np.float64)

    scale_p = np.tile(np.asarray(scale, np.float64), C)[:, None]
    vr = np.zeros((P, P + P), np.float64)
    vr[:, 0:P] = np.kron(np.eye(C), V)
    for c in range(C):
        rows = slice(c * N_INPUTS, (c + 1) * N_INPUTS)
        vr[rows, P + c] = 1.0
        vr[rows, P + C + c] = 1.0
        vr[rows, P + 64 + c] = lam * w[0] + b[0]
        vr[rows, P + 64 + C + c] = lam * w[1] + b[1]
    return (np.ascontiguousarray(scale_p.astype(np.float32)),
            np.ascontiguousarray(vr.astype(np.float32)))


def kernel(x, scale, theta, out_w, out_b, _trace=False):
    from concourse.bass_utils import run_bass_kernel_spmd

    x = np.ascontiguousarray(np.asarray(x, np.float32))
    scale_p, vr = _host_constants(scale, theta, out_w, out_b)

    in_maps = []
    for k in range(NCORES):
        xc = x[k * ROWS : (k + 1) * ROWS]
        xp = xc.reshape(C, NCOL, N_INPUTS).transpose(0, 2, 1).reshape(P, NCOL)
        xp = np.ascontiguousarray(np.concatenate([scale_p, xp], axis=1))
        in_maps.append({"xp": xp, "vr": vr})

    nc = _get_program()
    res = run_bass_kernel_spmd(
        nc, in_maps, core_ids=list(range(NCORES)), trace=_trace
    )
    parts = []
    for k in range(NCORES):
        op = res.results[k]["outp"]
        parts.append(np.stack([op[0:C].reshape(ROWS), op[C:].reshape(ROWS)], -1))
    out = np.concatenate(parts, axis=0)
    if _trace:
        return out, res
    return out

